# revision 11
# baseline (speedup 1.0000x reference)
"""Trainium2 Bass kernel for a GINE message-passing layer.

Reference computation (N=100000 nodes, E=600000 edges, D=128):
    msg  = relu(x[src] + edge_attr)            # [E, D]
    aggr = segment_sum(msg, dst, N)            # [N, D]
    z    = (1 + eps) * x + aggr
    h    = relu(bn1(z @ W1.T + b1)) @ W2.T + b2
    out  = relu(bn2(x + h))

Distribution strategy (8 NeuronCores, host-side shard/unshard):
  * Nodes are partitioned across the 8 cores (graph parallel) by a
    serpentine deal over in-degree-sorted nodes; within a core the same
    deal (plus a swap-repair pass) assigns nodes to 98 windows of 128 so
    every window receives at most 768 incoming edges (6 blocks of 128).
  * Edges are assigned to the core that owns their destination node, so
    the segment-sum is core-local.  The "halo" (src-node features) is
    materialized per (core, granule of 7 windows) as a compact
    feature-major bf16 table in HBM; each table is streamed sequentially
    into SBUF (SWDGE cast-DMA to f32) — no per-row DMA descriptors.
  * MLP weights / BN parameters are replicated (folded into per-feature
    affine scale+bias on the host; O(D) work).

Per-core device pipeline, bf16 activations (feature-major [feat, node]):
  1. per granule: stream the chunk table, then gather x[src] columns
     with GpSimd ap_gather (SBUF->SBUF, ~0.6ns/column — the SWDGE
     per-descriptor path costs ~8ns/row and was the old bottleneck),
  2. PE-transpose 3-block groups of the gathered feature-major columns
     into PSUM, VectorE adds the (edge-major) streamed edge_attr,
     ScalarE relu -> messages [edge, feat] in bf16,
  3. one-hot selection matrices S (VectorE iota-compare, bf16) turn the
     segment-sum into PE matmuls accumulated in PSUM:
         aggr[f, n] += sum_m msg[m, f] * S[m, n]
     plus an identity-matmul that adds (1+eps)*x (and transposes x to
     feature-major for free),
  4. MLP1 matmul + fused BN1+ReLU (ScalarE activation, per-partition
     affine), MLP2 matmul + identity-matmul residual + fused BN2+ReLU,
  5. output stays feature-major; the host transposes it back.
"""

import numpy as np
import ml_dtypes

import concourse.bass as bass
import concourse.bacc as bacc
import concourse.mybir as mybir
import concourse.tile as tile
from concourse.bass_utils import run_bass_kernel_spmd

# ---------------------------------------------------------------- constants
N_NODES = 100000
D = 128
P = 128                      # partitions
NCORES = 8
NW = 98                      # 128-node windows per core
BPC = NW * P                 # node slots per core (12544)
NPAD = NCORES * BPC          # padded node table rows (100352)
WG = 7                       # windows per granule (pipeline unit)
NG = NW // WG                # granules (14)
KB = 6                       # 128-edge blocks per window
WCAP = KB * P                # max in-edges per window (768)
NBG = WG * KB                # blocks per granule (42)
NBC = NW * KB                # blocks per core (588)
NELEM = NBG * P              # chunk-table rows per granule (5376)
HB = NBG // 2                # blocks per half-granule gather (21)
BN_EPS = 1e-5

BF16 = ml_dtypes.bfloat16

_NC_CACHE: dict = {}
LAST_RESULTS = None          # BassKernelResults of the most recent run


# ------------------------------------------------------------- host planning
def _serpentine(n, nbins):
    """Deal 0..n-1 into nbins bins, boustrophedon. Returns bin index."""
    g, o = np.divmod(np.arange(n), nbins)
    return np.where(g % 2 == 0, o, nbins - 1 - o)


def _plan_nodes(dst):
    """Serpentine deal of in-degree-sorted nodes to cores and windows,
    then swap-repair so every window has <= WCAP in-edges.
    Returns pos_of_node (global padded position)."""
    deg = np.bincount(dst, minlength=N_NODES)
    order = np.argsort(-deg, kind="stable")
    ranks = np.empty(N_NODES, np.int64)
    ranks[order] = np.arange(N_NODES)
    core_of = _serpentine(N_NODES, NCORES)[ranks]

    pos_of = np.empty(N_NODES, np.int64)
    for c in range(NCORES):
        nodes_c = order[core_of[order] == c]       # degree-desc within core
        nc_ = len(nodes_c)
        assert nc_ == N_NODES // NCORES and nc_ <= BPC
        w = _serpentine(nc_, NW)
        slot = np.arange(nc_) // NW        # swapped together with w below

        # swap-repair: windows must stay under WCAP in-edges
        dw = deg[nodes_c]
        cnt = np.bincount(w, weights=dw, minlength=NW).astype(np.int64)
        for _ in range(5000):
            hi = int(np.argmax(cnt))
            over = int(cnt[hi] - WCAP)
            if over <= 0:
                break
            done = False
            cand_hi = np.nonzero(w == hi)[0]
            dh = dw[cand_hi]
            for lo in np.argsort(cnt):
                lo = int(lo)
                slack = int(WCAP - cnt[lo])
                if slack <= 0 or lo == hi:
                    break
                t = min(over, slack)
                cand_lo = np.nonzero(w == lo)[0]
                dl = dw[cand_lo]
                dmat = dh[:, None] - dl[None, :]
                valid = (dmat >= 1) & (dmat <= slack)
                if valid.any():
                    score = np.where(valid, np.abs(dmat - t), 1 << 30)
                    ai, bi = np.unravel_index(np.argmin(score), score.shape)
                    a, b = cand_hi[ai], cand_lo[bi]
                    delta = int(dw[a] - dw[b])
                    w[a], w[b] = w[b], w[a]
                    slot[a], slot[b] = slot[b], slot[a]
                    cnt[hi] -= delta
                    cnt[lo] += delta
                    done = True
                    break
            if not done:
                raise RuntimeError("window repair failed")
        else:
            raise RuntimeError("window repair did not converge")
        assert cnt.max() <= WCAP
        pos_of[nodes_c] = c * BPC + w * P + slot
    return pos_of, core_of


# ------------------------------------------------------------- device build
def _build():
    """Build the per-core Bass program (SPMD: same program, per-core data)."""
    f32 = mybir.dt.float32
    bf16 = mybir.dt.bfloat16
    i16 = mybir.dt.int16
    NIDX = NG * (NBG * P // 16)          # idx columns (4704)

    nc = bacc.Bacc(None)
    xgt = nc.dram_tensor("xgt", [NG * P, NELEM], bf16, kind="ExternalInput")
    ea = nc.dram_tensor("ea", [P, NBC * D], bf16, kind="ExternalInput")
    xo = nc.dram_tensor("xo", [P, NW * D], bf16, kind="ExternalInput")
    idx = nc.dram_tensor("idx", [P, NIDX], i16, kind="ExternalInput")
    dstrel = nc.dram_tensor("dstrel", [P, NBC], bf16, kind="ExternalInput")
    iotac = nc.dram_tensor("iotac", [P, P], bf16, kind="ExternalInput")
    ideps = nc.dram_tensor("ideps", [P, P], bf16, kind="ExternalInput")
    iden = nc.dram_tensor("iden", [P, P], bf16, kind="ExternalInput")
    idenf = nc.dram_tensor("idenf", [P, P], f32, kind="ExternalInput")
    w1t = nc.dram_tensor("w1t", [D, D], bf16, kind="ExternalInput")
    w2t = nc.dram_tensor("w2t", [D, D], bf16, kind="ExternalInput")
    ab1 = nc.dram_tensor("ab1", [D, 2], f32, kind="ExternalInput")
    ab2 = nc.dram_tensor("ab2", [D, 2], f32, kind="ExternalInput")
    out = nc.dram_tensor("out", [P, NW * D], bf16, kind="ExternalOutput")

    relu = mybir.ActivationFunctionType.Relu
    copyf = mybir.ActivationFunctionType.Copy
    iseq = mybir.AluOpType.is_equal
    addop = mybir.AluOpType.add
    ts = bass.ts
    CPG = NBG * P // 16                   # idx cols per granule (336)
    CPH = CPG // 2                        # idx cols per half-granule (168)
    NIH = HB * P                          # idxs per half-granule call (2688)

    with tile.TileContext(nc) as tc:
        with (
            tc.tile_pool(name="const", bufs=1) as cp,
            tc.tile_pool(name="xck", bufs=2) as xckp,
            tc.tile_pool(name="gx", bufs=2) as gxp,
            tc.tile_pool(name="msg", bufs=2) as msgp,
            tc.tile_pool(name="ea", bufs=2) as eap,
            tc.tile_pool(name="sel", bufs=2) as sp,
            tc.tile_pool(name="xot", bufs=2) as xop,
            tc.tile_pool(name="z", bufs=3) as zp,
            tc.tile_pool(name="u", bufs=3) as up,
            tc.tile_pool(name="osb", bufs=2) as osp,
            tc.tile_pool(name="pt", bufs=2, space="PSUM") as ptp,
            tc.tile_pool(name="pz", bufs=2, space="PSUM") as pzp,
            tc.tile_pool(name="ph", bufs=2, space="PSUM") as php,
            tc.tile_pool(name="p2", bufs=2, space="PSUM") as p2p,
        ):
            # resident tensors
            idx_t = cp.tile([P, NIDX], i16)
            nc.sync.dma_start(out=idx_t[:, :], in_=idx[:, :])
            dst_t = cp.tile([P, NBC], bf16)
            nc.sync.dma_start(out=dst_t[:, :], in_=dstrel[:, :])
            iota_t = cp.tile([P, P], bf16)
            nc.sync.dma_start(out=iota_t[:, :], in_=iotac[:, :])
            ideps_t = cp.tile([P, P], bf16)
            nc.sync.dma_start(out=ideps_t[:, :], in_=ideps[:, :])
            iden_t = cp.tile([P, P], bf16)
            nc.sync.dma_start(out=iden_t[:, :], in_=iden[:, :])
            idenf_t = cp.tile([P, P], f32)
            nc.sync.dma_start(out=idenf_t[:, :], in_=idenf[:, :])
            w1t_t = cp.tile([D, D], bf16)
            nc.sync.dma_start(out=w1t_t[:, :], in_=w1t[:, :])
            w2t_t = cp.tile([D, D], bf16)
            nc.sync.dma_start(out=w2t_t[:, :], in_=w2t[:, :])
            ab1_t = cp.tile([D, 2], f32)
            nc.sync.dma_start(out=ab1_t[:, :], in_=ab1[:, :])
            ab2_t = cp.tile([D, 2], f32)
            nc.sync.dma_start(out=ab2_t[:, :], in_=ab2[:, :])

            for g in range(NG):
                # ---- stream this granule's compact src table (bf16->f32)
                xck = xckp.tile([P, NELEM], f32)
                nc.gpsimd.dma_start(
                    out=xck[:, :], in_=xgt[g * P:(g + 1) * P, :])
                # ---- edge attrs (edge-major) + own nodes
                ea_t = eap.tile([P, NBG * D], bf16)
                nc.sync.dma_start(
                    out=ea_t[:, :], in_=ea[:, g * NBG * D:(g + 1) * NBG * D])
                xot = xop.tile([P, WG * D], bf16)
                nc.sync.dma_start(
                    out=xot[:, :], in_=xo[:, g * WG * D:(g + 1) * WG * D])

                # ---- one-hot selection matrices for the whole granule
                sel = sp.tile([P, NBG * D], bf16)
                col0 = g * NBG
                in0 = (
                    dst_t[:, col0:col0 + NBG]
                    .rearrange("p (j o) -> p j o", o=1)
                    .to_broadcast([P, NBG, P])
                )
                in1 = (
                    iota_t[:, :]
                    .rearrange("p (o n) -> p o n", o=1)
                    .to_broadcast([P, NBG, P])
                )
                nc.vector.tensor_tensor(
                    out=sel[:, :].rearrange("p (j n) -> p j n", j=NBG),
                    in0=in0, in1=in1, op=iseq,
                )

                # ---- messages: gather columns, transpose, +ea, relu
                msg = msgp.tile([P, NBG * D], bf16)
                for h in range(2):
                    gx = gxp.tile([P, HB * D], f32)
                    nc.gpsimd.ap_gather(
                        gx[:, :].rearrange("p (n d) -> p n d", d=1),
                        xck[:, :].rearrange("p (n d) -> p n d", d=1),
                        idx_t[:, g * CPG + h * CPH:g * CPG + (h + 1) * CPH],
                        channels=P, num_elems=NELEM, d=1, num_idxs=NIH,
                    )
                    for q in range(HB // 3):
                        pt = ptp.tile([P, 3 * P], f32, space="PSUM")
                        for i in range(3):
                            nc.tensor.transpose(
                                pt[:, ts(i, P)],
                                gx[:, ts(q * 3 + i, P)],
                                idenf_t[:, :],
                            )
                        off = (h * HB + q * 3) * D
                        nc.vector.tensor_tensor(
                            out=msg[:, off:off + 3 * D], in0=pt[:, :],
                            in1=ea_t[:, off:off + 3 * D], op=addop,
                        )
                        nc.scalar.activation(
                            out=msg[:, off:off + 3 * D],
                            in_=msg[:, off:off + 3 * D], func=relu,
                        )

                osb = osp.tile([P, WG * D], bf16)
                for wi in range(WG):
                    # ---- aggregation: z[f,n] = (1+eps)x + sum(msg) in PSUM
                    pz = pzp.tile([P, P], f32, space="PSUM")
                    for k in range(KB):
                        b = wi * KB + k
                        nc.tensor.matmul(
                            out=pz[:, :], lhsT=msg[:, ts(b, D)],
                            rhs=sel[:, ts(b, D)],
                            start=(k == 0), stop=False,
                        )
                    nc.tensor.matmul(
                        out=pz[:, :], lhsT=xot[:, ts(wi, D)],
                        rhs=ideps_t[:, :], start=False, stop=True,
                    )
                    z = zp.tile([P, P], mybir.dt.bfloat16)
                    nc.scalar.activation(out=z[:, :], in_=pz[:, :], func=copyf)

                    # ---- MLP layer 1 + BN1 + relu
                    ph = php.tile([P, P], f32, space="PSUM")
                    nc.tensor.matmul(
                        out=ph[:, :], lhsT=w1t_t[:, :], rhs=z[:, :],
                        start=True, stop=True,
                    )
                    u = up.tile([P, P], mybir.dt.bfloat16)
                    nc.scalar.activation(
                        out=u[:, :], in_=ph[:, :], func=relu,
                        scale=ab1_t[:, 0:1], bias=ab1_t[:, 1:2],
                    )

                    # ---- MLP layer 2 + residual + BN2 + relu
                    p2 = p2p.tile([P, P], f32, space="PSUM")
                    nc.tensor.matmul(
                        out=p2[:, :], lhsT=w2t_t[:, :], rhs=u[:, :],
                        start=True, stop=False,
                    )
                    nc.tensor.matmul(
                        out=p2[:, :], lhsT=xot[:, ts(wi, D)],
                        rhs=iden_t[:, :], start=False, stop=True,
                    )
                    nc.scalar.activation(
                        out=osb[:, ts(wi, D)], in_=p2[:, :], func=relu,
                        scale=ab2_t[:, 0:1], bias=ab2_t[:, 1:2],
                    )

                nc.sync.dma_start(
                    out=out[:, g * WG * D:(g + 1) * WG * D], in_=osb[:, :]
                )

    nc.compile()
    return nc


def _get_nc(key):
    if key not in _NC_CACHE:
        _NC_CACHE[key] = _build()
    return _NC_CACHE[key]


# --------------------------------------------------------------- host driver
def _prepare(x, edge_index, edge_attr, eps, W1, b1, g1, bt1, rm1, rv1,
             W2, b2, g2, bt2, rm2, rv2):
    """Shard + reformat all inputs. Returns (in_maps, pos_of_node)."""
    src = np.asarray(edge_index[0], dtype=np.int64)
    dst = np.asarray(edge_index[1], dtype=np.int64)
    x = np.asarray(x, dtype=np.float32)
    ea_f = np.asarray(edge_attr, dtype=np.float32)

    pos_of, core_of = _plan_nodes(dst)
    x_bf = x.astype(BF16)

    # --- replicated constants
    epsf = float(np.asarray(eps))
    iotac = np.tile(np.arange(P, dtype=np.float32), (P, 1)).astype(BF16)
    ideps = ((1.0 + epsf) * np.eye(P)).astype(BF16)
    iden = np.eye(P, dtype=np.float32).astype(BF16)
    idenf = np.eye(P, dtype=np.float32)
    w1tm = np.ascontiguousarray(np.asarray(W1, np.float32).T).astype(BF16)
    w2tm = np.ascontiguousarray(np.asarray(W2, np.float32).T).astype(BF16)
    inv1 = 1.0 / np.sqrt(np.asarray(rv1, np.float32) + BN_EPS)
    a1 = np.asarray(g1, np.float32) * inv1
    beta1 = a1 * np.asarray(b1, np.float32) + np.asarray(bt1, np.float32) \
        - np.asarray(rm1, np.float32) * a1
    inv2 = 1.0 / np.sqrt(np.asarray(rv2, np.float32) + BN_EPS)
    a2 = np.asarray(g2, np.float32) * inv2
    beta2 = a2 * np.asarray(b2, np.float32) + np.asarray(bt2, np.float32) \
        - np.asarray(rm2, np.float32) * a2
    ab1 = np.ascontiguousarray(np.stack([a1, beta1], 1).astype(np.float32))
    ab2 = np.ascontiguousarray(np.stack([a2, beta2], 1).astype(np.float32))

    in_maps = []
    core_e = core_of[dst]
    for c in range(NCORES):
        em = core_e == c
        sc = src[em]
        posd = pos_of[dst[em]] - c * BPC
        wc = posd // P                      # window within core
        nrel = posd % P                     # node slot within window
        eac = ea_f[em]

        # edge -> slot: group by window (stable order)
        order_e = np.argsort(wc, kind="stable")
        ow = wc[order_e]
        counts = np.bincount(ow, minlength=NW)
        assert counts.max() <= WCAP
        starts = np.zeros(NW, np.int64)
        np.cumsum(counts[:-1], out=starts[1:])
        offs = np.arange(len(sc), dtype=np.int64) - starts[ow]
        k_o = offs // P
        q_o = offs % P
        col = ow * KB + k_o                 # global block col (0..NBC)

        so = sc[order_e]                    # src node per ordered edge
        # per-granule compact tables + chunk-local rows
        xgt_dev = np.zeros((NG * P, NELEM), dtype=BF16)
        srcrow = np.zeros((NBC, P), np.int16)
        gstart = np.zeros(NG + 1, np.int64)
        wstarts = np.concatenate([starts, [len(so)]])
        for g in range(NG):
            gstart[g] = wstarts[g * WG]
        gstart[NG] = len(so)
        for g in range(NG):
            seg = slice(gstart[g], gstart[g + 1])
            U, inv = np.unique(so[seg], return_inverse=True)
            assert len(U) <= NELEM
            xgt_dev[g * P:(g + 1) * P, :len(U)] = \
                np.ascontiguousarray(x_bf[U].T)
            srcrow[col[seg], q_o[seg]] = inv.astype(np.int16)

        dstrel_c = np.full((NBC, P), -1.0, np.float32)
        dstrel_c[col, q_o] = nrel[order_e].astype(np.float32)
        ea_cd = np.zeros((NBC, P, D), dtype=BF16)
        ea_cd[col, q_o] = eac[order_e].astype(BF16)

        # idx per half-granule: flat i = block*128 + q ;
        # sbuf[p, s] = flat[s*16 + p%16], replicated over 8 groups of 16
        idx_cols = []
        for g in range(NG):
            for h in range(2):
                b0 = g * NBG + h * HB
                flat = srcrow[b0:b0 + HB].reshape(-1)        # [HB*128]
                idx_cols.append(flat.reshape(-1, 16).T)      # [16, HB*8]
        idx16 = np.concatenate(idx_cols, axis=1)             # [16, NIDX]
        idx_dev = np.ascontiguousarray(np.tile(idx16, (P // 16, 1)))

        dstrel_dev = np.ascontiguousarray(dstrel_c.T).astype(BF16)
        ea_dev = np.ascontiguousarray(
            ea_cd.transpose(1, 0, 2).reshape(P, NBC * D))

        # own-node rows (window-major, node-major partitions)
        xperm = np.zeros((NW, P, D), dtype=BF16)
        nodes_c = np.nonzero(core_of == c)[0]
        pc = pos_of[nodes_c] - c * BPC
        xperm[pc // P, pc % P] = x_bf[nodes_c]
        xo_dev = np.ascontiguousarray(
            xperm.transpose(1, 0, 2).reshape(P, NW * D))

        in_maps.append({
            "xgt": xgt_dev,
            "ea": ea_dev,
            "xo": xo_dev,
            "idx": idx_dev,
            "dstrel": dstrel_dev,
            "iotac": iotac,
            "ideps": ideps,
            "iden": iden,
            "idenf": idenf,
            "w1t": w1tm,
            "w2t": w2tm,
            "ab1": ab1,
            "ab2": ab2,
        })
    return in_maps, pos_of


def kernel(**inputs) -> np.ndarray:
    global LAST_RESULTS
    x = np.asarray(inputs["x"], dtype=np.float32)
    assert x.shape == (N_NODES, D)

    in_maps, pos_of = _prepare(
        x, inputs["edge_index"], inputs["edge_attr_emb"], inputs["eps"],
        inputs["W1"], inputs["b1"], inputs["g1"], inputs["bt1"],
        inputs["rm1"], inputs["rv1"],
        inputs["W2"], inputs["b2"], inputs["g2"], inputs["bt2"],
        inputs["rm2"], inputs["rv2"],
    )
    nc = _get_nc("v3")
    res = run_bass_kernel_spmd(nc, in_maps, core_ids=list(range(NCORES)))
    LAST_RESULTS = res

    # out[c] is [P(feature), NW*P(node)]: col w*128+n -> padded node
    # row c*BPC + w*128 + n
    outp = np.stack([res.results[c]["out"] for c in range(NCORES)])
    out_rows = outp.astype(np.float32).reshape(NCORES, P, NW, P) \
        .transpose(0, 2, 3, 1).reshape(NPAD, D)
    return np.ascontiguousarray(out_rows[pos_of])


# revision 13
# speedup vs baseline: 1.1957x; 1.1957x over previous
"""Trainium2 Bass kernel for a GINE message-passing layer.

Reference computation (N=100000 nodes, E=600000 edges, D=128):
    msg  = relu(x[src] + edge_attr)            # [E, D]
    aggr = segment_sum(msg, dst, N)            # [N, D]
    z    = (1 + eps) * x + aggr
    h    = relu(bn1(z @ W1.T + b1)) @ W2.T + b2
    out  = relu(bn2(x + h))

Distribution strategy (8 NeuronCores, host-side shard/unshard):
  * Nodes are partitioned across the 8 cores (graph parallel) by a
    serpentine deal over in-degree-sorted nodes; within a core the same
    deal (plus a swap-repair pass) assigns nodes to 98 windows of 128 so
    every window receives at most 768 incoming edges (6 blocks of 128).
  * Edges are assigned to the core that owns their destination node, so
    the segment-sum is core-local.  The "halo" (src-node features) is
    materialized per (core, granule of 7 windows) as a compact
    feature-major bf16 table in HBM; each table is streamed sequentially
    into SBUF (SWDGE cast-DMA to f32) — no per-row DMA descriptors.
  * MLP weights / BN parameters are replicated (folded into per-feature
    affine scale+bias on the host; O(D) work).

Per-core device pipeline, bf16 activations (feature-major [feat, node]):
  1. per granule: stream the chunk table, then gather x[src] columns
     with GpSimd ap_gather (SBUF->SBUF, ~0.6ns/column — the SWDGE
     per-descriptor path costs ~8ns/row and was the old bottleneck),
  2. PE-transpose 3-block groups of the gathered feature-major columns
     into PSUM, VectorE adds the (edge-major) streamed edge_attr,
     ScalarE relu -> messages [edge, feat] in bf16,
  3. one-hot selection matrices S (VectorE iota-compare, bf16) turn the
     segment-sum into PE matmuls accumulated in PSUM:
         aggr[f, n] += sum_m msg[m, f] * S[m, n]
     plus an identity-matmul that adds (1+eps)*x (and transposes x to
     feature-major for free),
  4. MLP1 matmul + fused BN1+ReLU (ScalarE activation, per-partition
     affine), MLP2 matmul + identity-matmul residual + fused BN2+ReLU,
  5. output stays feature-major; the host transposes it back.
"""

import numpy as np
import ml_dtypes

import concourse.bass as bass
import concourse.bacc as bacc
import concourse.mybir as mybir
import concourse.tile as tile
from concourse.bass_utils import run_bass_kernel_spmd

# ---------------------------------------------------------------- constants
N_NODES = 100000
D = 128
P = 128                      # partitions
NCORES = 8
NW = 98                      # 128-node windows per core
BPC = NW * P                 # node slots per core (12544)
NPAD = NCORES * BPC          # padded node table rows (100352)
WG = 7                       # windows per granule (pipeline unit)
NG = NW // WG                # granules (14)
KB = 6                       # 128-edge blocks per window
WCAP = KB * P                # max in-edges per window (768)
NBG = WG * KB                # blocks per granule (42)
NBC = NW * KB                # blocks per core (588)
NELEM = NBG * P              # chunk-table rows per granule (5376)
HB = NBG // 2                # blocks per half-granule gather (21)
BN_EPS = 1e-5

BF16 = ml_dtypes.bfloat16

_NC_CACHE: dict = {}
LAST_RESULTS = None          # BassKernelResults of the most recent run


# ------------------------------------------------------------- host planning
def _serpentine(n, nbins):
    """Deal 0..n-1 into nbins bins, boustrophedon. Returns bin index."""
    g, o = np.divmod(np.arange(n), nbins)
    return np.where(g % 2 == 0, o, nbins - 1 - o)


def _plan_nodes(dst):
    """Serpentine deal of in-degree-sorted nodes to cores and windows,
    then swap-repair so every window has <= WCAP in-edges.
    Returns pos_of_node (global padded position)."""
    deg = np.bincount(dst, minlength=N_NODES)
    order = np.argsort(-deg, kind="stable")
    ranks = np.empty(N_NODES, np.int64)
    ranks[order] = np.arange(N_NODES)
    core_of = _serpentine(N_NODES, NCORES)[ranks]

    pos_of = np.empty(N_NODES, np.int64)
    for c in range(NCORES):
        nodes_c = order[core_of[order] == c]       # degree-desc within core
        nc_ = len(nodes_c)
        assert nc_ == N_NODES // NCORES and nc_ <= BPC
        w = _serpentine(nc_, NW)
        slot = np.arange(nc_) // NW        # swapped together with w below

        # swap-repair: windows must stay under WCAP in-edges
        dw = deg[nodes_c]
        cnt = np.bincount(w, weights=dw, minlength=NW).astype(np.int64)
        for _ in range(5000):
            hi = int(np.argmax(cnt))
            over = int(cnt[hi] - WCAP)
            if over <= 0:
                break
            done = False
            cand_hi = np.nonzero(w == hi)[0]
            dh = dw[cand_hi]
            for lo in np.argsort(cnt):
                lo = int(lo)
                slack = int(WCAP - cnt[lo])
                if slack <= 0 or lo == hi:
                    break
                t = min(over, slack)
                cand_lo = np.nonzero(w == lo)[0]
                dl = dw[cand_lo]
                dmat = dh[:, None] - dl[None, :]
                valid = (dmat >= 1) & (dmat <= slack)
                if valid.any():
                    score = np.where(valid, np.abs(dmat - t), 1 << 30)
                    ai, bi = np.unravel_index(np.argmin(score), score.shape)
                    a, b = cand_hi[ai], cand_lo[bi]
                    delta = int(dw[a] - dw[b])
                    w[a], w[b] = w[b], w[a]
                    slot[a], slot[b] = slot[b], slot[a]
                    cnt[hi] -= delta
                    cnt[lo] += delta
                    done = True
                    break
            if not done:
                raise RuntimeError("window repair failed")
        else:
            raise RuntimeError("window repair did not converge")
        assert cnt.max() <= WCAP
        pos_of[nodes_c] = c * BPC + w * P + slot
    return pos_of, core_of


# ------------------------------------------------------------- device build
def _build():
    """Build the per-core Bass program (SPMD: same program, per-core data)."""
    f32 = mybir.dt.float32
    bf16 = mybir.dt.bfloat16
    i16 = mybir.dt.int16
    NIDX = NG * (NBG * P // 16)          # idx columns (4704)

    nc = bacc.Bacc(None)
    xgt = nc.dram_tensor("xgt", [NG * P, NELEM], f32, kind="ExternalInput")
    ea = nc.dram_tensor("ea", [P, NBC * D], bf16, kind="ExternalInput")
    xo = nc.dram_tensor("xo", [P, NW * D], bf16, kind="ExternalInput")
    idx = nc.dram_tensor("idx", [P, NIDX], i16, kind="ExternalInput")
    dstrel = nc.dram_tensor("dstrel", [P, NBC], bf16, kind="ExternalInput")
    iotac = nc.dram_tensor("iotac", [P, P], bf16, kind="ExternalInput")
    ideps = nc.dram_tensor("ideps", [P, P], bf16, kind="ExternalInput")
    iden = nc.dram_tensor("iden", [P, P], bf16, kind="ExternalInput")
    w1t = nc.dram_tensor("w1t", [D, D], bf16, kind="ExternalInput")
    w2t = nc.dram_tensor("w2t", [D, D], bf16, kind="ExternalInput")
    ab1 = nc.dram_tensor("ab1", [D, 2], f32, kind="ExternalInput")
    ab2 = nc.dram_tensor("ab2", [D, 2], f32, kind="ExternalInput")
    out = nc.dram_tensor("out", [P, NW * D], bf16, kind="ExternalOutput")

    relu = mybir.ActivationFunctionType.Relu
    copyf = mybir.ActivationFunctionType.Copy
    iseq = mybir.AluOpType.is_equal
    addop = mybir.AluOpType.add
    ts = bass.ts
    CPG = NBG * P // 16                   # idx cols per granule (336)
    CPH = CPG // 2                        # idx cols per half-granule (168)
    NIH = HB * P                          # idxs per half-granule call (2688)

    with tile.TileContext(nc) as tc:
        with (
            tc.tile_pool(name="const", bufs=1) as cp,
            tc.tile_pool(name="xck", bufs=2) as xckp,
            tc.tile_pool(name="gx", bufs=2) as gxp,
            tc.tile_pool(name="msg", bufs=2) as msgp,
            tc.tile_pool(name="mfm", bufs=2) as mfmp,
            tc.tile_pool(name="ea", bufs=2) as eap,
            tc.tile_pool(name="sel", bufs=2) as sp,
            tc.tile_pool(name="xot", bufs=2) as xop,
            tc.tile_pool(name="z", bufs=3) as zp,
            tc.tile_pool(name="u", bufs=3) as up,
            tc.tile_pool(name="osb", bufs=2) as osp,
            tc.tile_pool(name="pt", bufs=2, space="PSUM") as ptp,
            tc.tile_pool(name="pz", bufs=2, space="PSUM") as pzp,
            tc.tile_pool(name="ph", bufs=2, space="PSUM") as php,
            tc.tile_pool(name="p2", bufs=2, space="PSUM") as p2p,
        ):
            # resident tensors
            idx_t = cp.tile([P, NIDX], i16)
            nc.sync.dma_start(out=idx_t[:, :], in_=idx[:, :])
            dst_t = cp.tile([P, NBC], bf16)
            nc.sync.dma_start(out=dst_t[:, :], in_=dstrel[:, :])
            iota_t = cp.tile([P, P], bf16)
            nc.sync.dma_start(out=iota_t[:, :], in_=iotac[:, :])
            ideps_t = cp.tile([P, P], bf16)
            nc.sync.dma_start(out=ideps_t[:, :], in_=ideps[:, :])
            iden_t = cp.tile([P, P], bf16)
            nc.sync.dma_start(out=iden_t[:, :], in_=iden[:, :])
            w1t_t = cp.tile([D, D], bf16)
            nc.sync.dma_start(out=w1t_t[:, :], in_=w1t[:, :])
            w2t_t = cp.tile([D, D], bf16)
            nc.sync.dma_start(out=w2t_t[:, :], in_=w2t[:, :])
            ab1_t = cp.tile([D, 2], f32)
            nc.sync.dma_start(out=ab1_t[:, :], in_=ab1[:, :])
            ab2_t = cp.tile([D, 2], f32)
            nc.sync.dma_start(out=ab2_t[:, :], in_=ab2[:, :])

            for g in range(NG):
                # ---- stream this granule's compact src table (f32)
                xck = xckp.tile([P, NELEM], f32)
                nc.sync.dma_start(
                    out=xck[:, :], in_=xgt[g * P:(g + 1) * P, :])
                # ---- edge attrs (edge-major) + own nodes
                ea_t = eap.tile([P, NBG * D], bf16)
                nc.sync.dma_start(
                    out=ea_t[:, :], in_=ea[:, g * NBG * D:(g + 1) * NBG * D])
                xot = xop.tile([P, WG * D], bf16)
                nc.sync.dma_start(
                    out=xot[:, :], in_=xo[:, g * WG * D:(g + 1) * WG * D])

                # ---- one-hot selection matrices for the whole granule
                sel = sp.tile([P, NBG * D], bf16)
                col0 = g * NBG
                in0 = (
                    dst_t[:, col0:col0 + NBG]
                    .rearrange("p (j o) -> p j o", o=1)
                    .to_broadcast([P, NBG, P])
                )
                in1 = (
                    iota_t[:, :]
                    .rearrange("p (o n) -> p o n", o=1)
                    .to_broadcast([P, NBG, P])
                )
                nc.vector.tensor_tensor(
                    out=sel[:, :].rearrange("p (j n) -> p j n", j=NBG),
                    in0=in0, in1=in1, op=iseq,
                )

                # ---- messages: gather columns (feature-major), +ea on
                # VectorE, PE-transpose 3-block groups, ScalarE relu does
                # the PSUM->SBUF move into edge-major msg
                msg = msgp.tile([P, NBG * D], bf16)
                for h in range(2):
                    gx = gxp.tile([P, HB * D], f32)
                    nc.gpsimd.ap_gather(
                        gx[:, :].rearrange("p (n d) -> p n d", d=1),
                        xck[:, :].rearrange("p (n d) -> p n d", d=1),
                        idx_t[:, g * CPG + h * CPH:g * CPG + (h + 1) * CPH],
                        channels=P, num_elems=NELEM, d=1, num_idxs=NIH,
                    )
                    mfm = mfmp.tile([P, HB * D], bf16)
                    nc.vector.tensor_tensor(
                        out=mfm[:, :], in0=gx[:, :],
                        in1=ea_t[:, h * HB * D:(h + 1) * HB * D], op=addop,
                    )
                    for q in range(HB // 3):
                        pt = ptp.tile([P, 3 * P], bf16, space="PSUM")
                        for i in range(3):
                            nc.tensor.transpose(
                                pt[:, ts(i, P)],
                                mfm[:, ts(q * 3 + i, P)],
                                iden_t[:, :],
                            )
                        off = (h * HB + q * 3) * D
                        nc.scalar.activation(
                            out=msg[:, off:off + 3 * D],
                            in_=pt[:, :], func=relu,
                        )

                osb = osp.tile([P, WG * D], bf16)
                for wi in range(WG):
                    # ---- aggregation: z[f,n] = (1+eps)x + sum(msg) in PSUM
                    pz = pzp.tile([P, P], f32, space="PSUM")
                    for k in range(KB):
                        b = wi * KB + k
                        nc.tensor.matmul(
                            out=pz[:, :], lhsT=msg[:, ts(b, D)],
                            rhs=sel[:, ts(b, D)],
                            start=(k == 0), stop=False,
                        )
                    nc.tensor.matmul(
                        out=pz[:, :], lhsT=xot[:, ts(wi, D)],
                        rhs=ideps_t[:, :], start=False, stop=True,
                    )
                    z = zp.tile([P, P], mybir.dt.bfloat16)
                    nc.scalar.activation(out=z[:, :], in_=pz[:, :], func=copyf)

                    # ---- MLP layer 1 + BN1 + relu
                    ph = php.tile([P, P], f32, space="PSUM")
                    nc.tensor.matmul(
                        out=ph[:, :], lhsT=w1t_t[:, :], rhs=z[:, :],
                        start=True, stop=True,
                    )
                    u = up.tile([P, P], mybir.dt.bfloat16)
                    nc.scalar.activation(
                        out=u[:, :], in_=ph[:, :], func=relu,
                        scale=ab1_t[:, 0:1], bias=ab1_t[:, 1:2],
                    )

                    # ---- MLP layer 2 + residual + BN2 + relu
                    p2 = p2p.tile([P, P], f32, space="PSUM")
                    nc.tensor.matmul(
                        out=p2[:, :], lhsT=w2t_t[:, :], rhs=u[:, :],
                        start=True, stop=False,
                    )
                    nc.tensor.matmul(
                        out=p2[:, :], lhsT=xot[:, ts(wi, D)],
                        rhs=iden_t[:, :], start=False, stop=True,
                    )
                    nc.scalar.activation(
                        out=osb[:, ts(wi, D)], in_=p2[:, :], func=relu,
                        scale=ab2_t[:, 0:1], bias=ab2_t[:, 1:2],
                    )

                nc.sync.dma_start(
                    out=out[:, g * WG * D:(g + 1) * WG * D], in_=osb[:, :]
                )

    nc.compile()
    return nc


def _get_nc(key):
    if key not in _NC_CACHE:
        _NC_CACHE[key] = _build()
    return _NC_CACHE[key]


# --------------------------------------------------------------- host driver
def _prepare(x, edge_index, edge_attr, eps, W1, b1, g1, bt1, rm1, rv1,
             W2, b2, g2, bt2, rm2, rv2):
    """Shard + reformat all inputs. Returns (in_maps, pos_of_node)."""
    src = np.asarray(edge_index[0], dtype=np.int64)
    dst = np.asarray(edge_index[1], dtype=np.int64)
    x = np.asarray(x, dtype=np.float32)
    ea_f = np.asarray(edge_attr, dtype=np.float32)

    pos_of, core_of = _plan_nodes(dst)
    x_bf = x.astype(BF16)

    # --- replicated constants
    epsf = float(np.asarray(eps))
    iotac = np.tile(np.arange(P, dtype=np.float32), (P, 1)).astype(BF16)
    ideps = ((1.0 + epsf) * np.eye(P)).astype(BF16)
    iden = np.eye(P, dtype=np.float32).astype(BF16)
    w1tm = np.ascontiguousarray(np.asarray(W1, np.float32).T).astype(BF16)
    w2tm = np.ascontiguousarray(np.asarray(W2, np.float32).T).astype(BF16)
    inv1 = 1.0 / np.sqrt(np.asarray(rv1, np.float32) + BN_EPS)
    a1 = np.asarray(g1, np.float32) * inv1
    beta1 = a1 * np.asarray(b1, np.float32) + np.asarray(bt1, np.float32) \
        - np.asarray(rm1, np.float32) * a1
    inv2 = 1.0 / np.sqrt(np.asarray(rv2, np.float32) + BN_EPS)
    a2 = np.asarray(g2, np.float32) * inv2
    beta2 = a2 * np.asarray(b2, np.float32) + np.asarray(bt2, np.float32) \
        - np.asarray(rm2, np.float32) * a2
    ab1 = np.ascontiguousarray(np.stack([a1, beta1], 1).astype(np.float32))
    ab2 = np.ascontiguousarray(np.stack([a2, beta2], 1).astype(np.float32))

    in_maps = []
    core_e = core_of[dst]
    for c in range(NCORES):
        em = core_e == c
        sc = src[em]
        posd = pos_of[dst[em]] - c * BPC
        wc = posd // P                      # window within core
        nrel = posd % P                     # node slot within window
        eac = ea_f[em]

        # edge -> slot: group by window (stable order)
        order_e = np.argsort(wc, kind="stable")
        ow = wc[order_e]
        counts = np.bincount(ow, minlength=NW)
        assert counts.max() <= WCAP
        starts = np.zeros(NW, np.int64)
        np.cumsum(counts[:-1], out=starts[1:])
        offs = np.arange(len(sc), dtype=np.int64) - starts[ow]
        k_o = offs // P
        q_o = offs % P
        col = ow * KB + k_o                 # global block col (0..NBC)

        so = sc[order_e]                    # src node per ordered edge
        # per-granule compact tables + chunk-local rows
        xgt_dev = np.zeros((NG * P, NELEM), dtype=np.float32)
        srcrow = np.zeros((NBC, P), np.int16)
        gstart = np.zeros(NG + 1, np.int64)
        wstarts = np.concatenate([starts, [len(so)]])
        for g in range(NG):
            gstart[g] = wstarts[g * WG]
        gstart[NG] = len(so)
        for g in range(NG):
            seg = slice(gstart[g], gstart[g + 1])
            U, inv = np.unique(so[seg], return_inverse=True)
            assert len(U) <= NELEM
            xgt_dev[g * P:(g + 1) * P, :len(U)] = \
                np.ascontiguousarray(x_bf[U].astype(np.float32).T)
            srcrow[col[seg], q_o[seg]] = inv.astype(np.int16)

        dstrel_c = np.full((NBC, P), -1.0, np.float32)
        dstrel_c[col, q_o] = nrel[order_e].astype(np.float32)
        ea_cd = np.zeros((NBC, P, D), dtype=BF16)
        ea_cd[col, q_o] = eac[order_e].astype(BF16)

        # idx per half-granule: flat i = block*128 + q ;
        # sbuf[p, s] = flat[s*16 + p%16], replicated over 8 groups of 16
        idx_cols = []
        for g in range(NG):
            for h in range(2):
                b0 = g * NBG + h * HB
                flat = srcrow[b0:b0 + HB].reshape(-1)        # [HB*128]
                idx_cols.append(flat.reshape(-1, 16).T)      # [16, HB*8]
        idx16 = np.concatenate(idx_cols, axis=1)             # [16, NIDX]
        idx_dev = np.ascontiguousarray(np.tile(idx16, (P // 16, 1)))

        dstrel_dev = np.ascontiguousarray(dstrel_c.T).astype(BF16)
        # feature-major: ea_dev[f, col*128 + q] = ea[edge at (col, q)][f]
        ea_dev = np.ascontiguousarray(
            ea_cd.transpose(2, 0, 1).reshape(P, NBC * P))

        # own-node rows (window-major, node-major partitions)
        xperm = np.zeros((NW, P, D), dtype=BF16)
        nodes_c = np.nonzero(core_of == c)[0]
        pc = pos_of[nodes_c] - c * BPC
        xperm[pc // P, pc % P] = x_bf[nodes_c]
        xo_dev = np.ascontiguousarray(
            xperm.transpose(1, 0, 2).reshape(P, NW * D))

        in_maps.append({
            "xgt": xgt_dev,
            "ea": ea_dev,
            "xo": xo_dev,
            "idx": idx_dev,
            "dstrel": dstrel_dev,
            "iotac": iotac,
            "ideps": ideps,
            "iden": iden,
            "w1t": w1tm,
            "w2t": w2tm,
            "ab1": ab1,
            "ab2": ab2,
        })
    return in_maps, pos_of


def kernel(**inputs) -> np.ndarray:
    global LAST_RESULTS
    x = np.asarray(inputs["x"], dtype=np.float32)
    assert x.shape == (N_NODES, D)

    in_maps, pos_of = _prepare(
        x, inputs["edge_index"], inputs["edge_attr_emb"], inputs["eps"],
        inputs["W1"], inputs["b1"], inputs["g1"], inputs["bt1"],
        inputs["rm1"], inputs["rv1"],
        inputs["W2"], inputs["b2"], inputs["g2"], inputs["bt2"],
        inputs["rm2"], inputs["rv2"],
    )
    nc = _get_nc("v3")
    res = run_bass_kernel_spmd(nc, in_maps, core_ids=list(range(NCORES)))
    LAST_RESULTS = res

    # out[c] is [P(feature), NW*P(node)]: col w*128+n -> padded node
    # row c*BPC + w*128 + n
    outp = np.stack([res.results[c]["out"] for c in range(NCORES)])
    out_rows = outp.astype(np.float32).reshape(NCORES, P, NW, P) \
        .transpose(0, 2, 3, 1).reshape(NPAD, D)
    return np.ascontiguousarray(out_rows[pos_of])


# revision 14
# speedup vs baseline: 1.1968x; 1.0009x over previous
"""Trainium2 Bass kernel for a GINE message-passing layer.

Reference computation (N=100000 nodes, E=600000 edges, D=128):
    msg  = relu(x[src] + edge_attr)            # [E, D]
    aggr = segment_sum(msg, dst, N)            # [N, D]
    z    = (1 + eps) * x + aggr
    h    = relu(bn1(z @ W1.T + b1)) @ W2.T + b2
    out  = relu(bn2(x + h))

Distribution strategy (8 NeuronCores, host-side shard/unshard):
  * Nodes are partitioned across the 8 cores (graph parallel) by a
    serpentine deal over in-degree-sorted nodes; within a core the same
    deal (plus a swap-repair pass) assigns nodes to 98 windows of 128 so
    every window receives at most 768 incoming edges (6 blocks of 128).
  * Edges are assigned to the core that owns their destination node, so
    the segment-sum is core-local.  The "halo" (src-node features) is
    materialized per (core, granule of 7 windows) as a compact
    feature-major bf16 table in HBM; each table is streamed sequentially
    into SBUF (SWDGE cast-DMA to f32) — no per-row DMA descriptors.
  * MLP weights / BN parameters are replicated (folded into per-feature
    affine scale+bias on the host; O(D) work).

Per-core device pipeline, bf16 activations (feature-major [feat, node]):
  1. per granule: stream the chunk table, then gather x[src] columns
     with GpSimd ap_gather (SBUF->SBUF, ~0.6ns/column — the SWDGE
     per-descriptor path costs ~8ns/row and was the old bottleneck),
  2. PE-transpose 3-block groups of the gathered feature-major columns
     into PSUM, VectorE adds the (edge-major) streamed edge_attr,
     ScalarE relu -> messages [edge, feat] in bf16,
  3. one-hot selection matrices S (VectorE iota-compare, bf16) turn the
     segment-sum into PE matmuls accumulated in PSUM:
         aggr[f, n] += sum_m msg[m, f] * S[m, n]
     plus an identity-matmul that adds (1+eps)*x (and transposes x to
     feature-major for free),
  4. MLP1 matmul + fused BN1+ReLU (ScalarE activation, per-partition
     affine), MLP2 matmul + identity-matmul residual + fused BN2+ReLU,
  5. output stays feature-major; the host transposes it back.
"""

import numpy as np
import ml_dtypes

import concourse.bass as bass
import concourse.bacc as bacc
import concourse.mybir as mybir
import concourse.tile as tile
from concourse.bass_utils import run_bass_kernel_spmd

# ---------------------------------------------------------------- constants
N_NODES = 100000
D = 128
P = 128                      # partitions
NCORES = 8
NW = 98                      # 128-node windows per core
BPC = NW * P                 # node slots per core (12544)
NPAD = NCORES * BPC          # padded node table rows (100352)
WG = 7                       # windows per granule (pipeline unit)
NG = NW // WG                # granules (14)
KB = 6                       # 128-edge blocks per window
WCAP = KB * P                # max in-edges per window (768)
NBG = WG * KB                # blocks per granule (42)
NBC = NW * KB                # blocks per core (588)
NELEM = NBG * P              # chunk-table rows per granule (5376)
HB = NBG // 2                # blocks per half-granule gather (21)
BN_EPS = 1e-5

BF16 = ml_dtypes.bfloat16

_NC_CACHE: dict = {}
LAST_RESULTS = None          # BassKernelResults of the most recent run


# ------------------------------------------------------------- host planning
def _serpentine(n, nbins):
    """Deal 0..n-1 into nbins bins, boustrophedon. Returns bin index."""
    g, o = np.divmod(np.arange(n), nbins)
    return np.where(g % 2 == 0, o, nbins - 1 - o)


def _plan_nodes(dst):
    """Serpentine deal of in-degree-sorted nodes to cores and windows,
    then swap-repair so every window has <= WCAP in-edges.
    Returns pos_of_node (global padded position)."""
    deg = np.bincount(dst, minlength=N_NODES)
    order = np.argsort(-deg, kind="stable")
    ranks = np.empty(N_NODES, np.int64)
    ranks[order] = np.arange(N_NODES)
    core_of = _serpentine(N_NODES, NCORES)[ranks]

    pos_of = np.empty(N_NODES, np.int64)
    for c in range(NCORES):
        nodes_c = order[core_of[order] == c]       # degree-desc within core
        nc_ = len(nodes_c)
        assert nc_ == N_NODES // NCORES and nc_ <= BPC
        w = _serpentine(nc_, NW)
        slot = np.arange(nc_) // NW        # swapped together with w below

        # swap-repair: windows must stay under WCAP in-edges
        dw = deg[nodes_c]
        cnt = np.bincount(w, weights=dw, minlength=NW).astype(np.int64)
        for _ in range(5000):
            hi = int(np.argmax(cnt))
            over = int(cnt[hi] - WCAP)
            if over <= 0:
                break
            done = False
            cand_hi = np.nonzero(w == hi)[0]
            dh = dw[cand_hi]
            for lo in np.argsort(cnt):
                lo = int(lo)
                slack = int(WCAP - cnt[lo])
                if slack <= 0 or lo == hi:
                    break
                t = min(over, slack)
                cand_lo = np.nonzero(w == lo)[0]
                dl = dw[cand_lo]
                dmat = dh[:, None] - dl[None, :]
                valid = (dmat >= 1) & (dmat <= slack)
                if valid.any():
                    score = np.where(valid, np.abs(dmat - t), 1 << 30)
                    ai, bi = np.unravel_index(np.argmin(score), score.shape)
                    a, b = cand_hi[ai], cand_lo[bi]
                    delta = int(dw[a] - dw[b])
                    w[a], w[b] = w[b], w[a]
                    slot[a], slot[b] = slot[b], slot[a]
                    cnt[hi] -= delta
                    cnt[lo] += delta
                    done = True
                    break
            if not done:
                raise RuntimeError("window repair failed")
        else:
            raise RuntimeError("window repair did not converge")
        assert cnt.max() <= WCAP
        pos_of[nodes_c] = c * BPC + w * P + slot
    return pos_of, core_of


# ------------------------------------------------------------- device build
def _build():
    """Build the per-core Bass program (SPMD: same program, per-core data)."""
    f32 = mybir.dt.float32
    bf16 = mybir.dt.bfloat16
    i16 = mybir.dt.int16
    NIDX = NG * (NBG * P // 16)          # idx columns (4704)

    nc = bacc.Bacc(None)
    xgt = nc.dram_tensor("xgt", [NG * P, NELEM], f32, kind="ExternalInput")
    ea = nc.dram_tensor("ea", [P, NBC * D], bf16, kind="ExternalInput")
    xo = nc.dram_tensor("xo", [P, NW * D], bf16, kind="ExternalInput")
    idx = nc.dram_tensor("idx", [P, NIDX], i16, kind="ExternalInput")
    dstrel = nc.dram_tensor("dstrel", [P, NBC], bf16, kind="ExternalInput")
    iotac = nc.dram_tensor("iotac", [P, P], bf16, kind="ExternalInput")
    ideps = nc.dram_tensor("ideps", [P, P], bf16, kind="ExternalInput")
    iden = nc.dram_tensor("iden", [P, P], bf16, kind="ExternalInput")
    w1t = nc.dram_tensor("w1t", [D, D], bf16, kind="ExternalInput")
    w2t = nc.dram_tensor("w2t", [D, D], bf16, kind="ExternalInput")
    ab1 = nc.dram_tensor("ab1", [D, 2], f32, kind="ExternalInput")
    ab2 = nc.dram_tensor("ab2", [D, 2], f32, kind="ExternalInput")
    out = nc.dram_tensor("out", [P, NW * D], bf16, kind="ExternalOutput")

    relu = mybir.ActivationFunctionType.Relu
    copyf = mybir.ActivationFunctionType.Copy
    iseq = mybir.AluOpType.is_equal
    addop = mybir.AluOpType.add
    ts = bass.ts
    CPG = NBG * P // 16                   # idx cols per granule (336)
    CPH = CPG // 2                        # idx cols per half-granule (168)
    NIH = HB * P                          # idxs per half-granule call (2688)

    with tile.TileContext(nc) as tc:
        with (
            tc.tile_pool(name="const", bufs=1) as cp,
            tc.tile_pool(name="xck", bufs=2) as xckp,
            tc.tile_pool(name="gx", bufs=2) as gxp,
            tc.tile_pool(name="msg", bufs=2) as msgp,
            tc.tile_pool(name="mfm", bufs=2) as mfmp,
            tc.tile_pool(name="ea", bufs=2) as eap,
            tc.tile_pool(name="sel", bufs=2) as sp,
            tc.tile_pool(name="xot", bufs=2) as xop,
            tc.tile_pool(name="z", bufs=3) as zp,
            tc.tile_pool(name="u", bufs=3) as up,
            tc.tile_pool(name="osb", bufs=2) as osp,
            tc.tile_pool(name="pt", bufs=2, space="PSUM") as ptp,
            tc.tile_pool(name="pz", bufs=2, space="PSUM") as pzp,
            tc.tile_pool(name="ph", bufs=2, space="PSUM") as php,
            tc.tile_pool(name="p2", bufs=2, space="PSUM") as p2p,
        ):
            # resident tensors
            idx_t = cp.tile([P, NIDX], i16)
            nc.sync.dma_start(out=idx_t[:, :], in_=idx[:, :])
            dst_t = cp.tile([P, NBC], bf16)
            nc.sync.dma_start(out=dst_t[:, :], in_=dstrel[:, :])
            iota_t = cp.tile([P, P], bf16)
            nc.sync.dma_start(out=iota_t[:, :], in_=iotac[:, :])
            ideps_t = cp.tile([P, P], bf16)
            nc.sync.dma_start(out=ideps_t[:, :], in_=ideps[:, :])
            iden_t = cp.tile([P, P], bf16)
            nc.sync.dma_start(out=iden_t[:, :], in_=iden[:, :])
            w1t_t = cp.tile([D, D], bf16)
            nc.sync.dma_start(out=w1t_t[:, :], in_=w1t[:, :])
            w2t_t = cp.tile([D, D], bf16)
            nc.sync.dma_start(out=w2t_t[:, :], in_=w2t[:, :])
            ab1_t = cp.tile([D, 2], f32)
            nc.sync.dma_start(out=ab1_t[:, :], in_=ab1[:, :])
            ab2_t = cp.tile([D, 2], f32)
            nc.sync.dma_start(out=ab2_t[:, :], in_=ab2[:, :])

            for g in range(NG):
                # ---- stream this granule's compact src table (f32)
                xck = xckp.tile([P, NELEM], f32)
                nc.sync.dma_start(
                    out=xck[:, :], in_=xgt[g * P:(g + 1) * P, :])
                # ---- edge attrs (edge-major) + own nodes
                ea_t = eap.tile([P, NBG * D], bf16)
                nc.sync.dma_start(
                    out=ea_t[:, :], in_=ea[:, g * NBG * D:(g + 1) * NBG * D])
                xot = xop.tile([P, WG * D], bf16)
                nc.sync.dma_start(
                    out=xot[:, :], in_=xo[:, g * WG * D:(g + 1) * WG * D])

                # ---- one-hot selection matrices for the whole granule
                sel = sp.tile([P, NBG * D], bf16)
                col0 = g * NBG
                in0 = (
                    dst_t[:, col0:col0 + NBG]
                    .rearrange("p (j o) -> p j o", o=1)
                    .to_broadcast([P, NBG, P])
                )
                in1 = (
                    iota_t[:, :]
                    .rearrange("p (o n) -> p o n", o=1)
                    .to_broadcast([P, NBG, P])
                )
                nc.vector.tensor_tensor(
                    out=sel[:, :].rearrange("p (j n) -> p j n", j=NBG),
                    in0=in0, in1=in1, op=iseq,
                )

                # ---- messages: gather columns (feature-major), +ea on
                # VectorE, PE-transpose 3-block groups, ScalarE relu does
                # the PSUM->SBUF move into edge-major msg
                msg = msgp.tile([P, NBG * D], bf16)
                for h in range(2):
                    gx = gxp.tile([P, HB * D], f32)
                    nc.gpsimd.ap_gather(
                        gx[:, :].rearrange("p (n d) -> p n d", d=1),
                        xck[:, :].rearrange("p (n d) -> p n d", d=1),
                        idx_t[:, g * CPG + h * CPH:g * CPG + (h + 1) * CPH],
                        channels=P, num_elems=NELEM, d=1, num_idxs=NIH,
                    )
                    mfm = mfmp.tile([P, HB * D], bf16)
                    nc.vector.tensor_tensor(
                        out=mfm[:, :], in0=gx[:, :],
                        in1=ea_t[:, h * HB * D:(h + 1) * HB * D], op=addop,
                    )
                    for q in range(HB // 3):
                        pt = ptp.tile([P, 3 * P], bf16, space="PSUM")
                        for i in range(3):
                            nc.tensor.transpose(
                                pt[:, ts(i, P)],
                                mfm[:, ts(q * 3 + i, P)],
                                iden_t[:, :],
                            )
                        off = (h * HB + q * 3) * D
                        nc.scalar.activation(
                            out=msg[:, off:off + 3 * D],
                            in_=pt[:, :], func=relu,
                        )

                osb = osp.tile([P, WG * D], bf16)
                for wi in range(WG):
                    # ---- aggregation: z[f,n] = (1+eps)x + sum(msg) in PSUM
                    pz = pzp.tile([P, P], f32, space="PSUM")
                    for k in range(KB):
                        b = wi * KB + k
                        nc.tensor.matmul(
                            out=pz[:, :], lhsT=msg[:, ts(b, D)],
                            rhs=sel[:, ts(b, D)],
                            start=(k == 0), stop=False,
                        )
                    nc.tensor.matmul(
                        out=pz[:, :], lhsT=xot[:, ts(wi, D)],
                        rhs=ideps_t[:, :], start=False, stop=True,
                    )
                    z = zp.tile([P, P], mybir.dt.bfloat16)
                    nc.scalar.activation(out=z[:, :], in_=pz[:, :], func=copyf)

                    # ---- MLP layer 1 + BN1 + relu
                    ph = php.tile([P, P], f32, space="PSUM")
                    nc.tensor.matmul(
                        out=ph[:, :], lhsT=w1t_t[:, :], rhs=z[:, :],
                        start=True, stop=True,
                    )
                    u = up.tile([P, P], mybir.dt.bfloat16)
                    nc.scalar.activation(
                        out=u[:, :], in_=ph[:, :], func=relu,
                        scale=ab1_t[:, 0:1], bias=ab1_t[:, 1:2],
                    )

                    # ---- MLP layer 2 + residual + BN2 + relu
                    p2 = p2p.tile([P, P], f32, space="PSUM")
                    nc.tensor.matmul(
                        out=p2[:, :], lhsT=w2t_t[:, :], rhs=u[:, :],
                        start=True, stop=False,
                    )
                    nc.tensor.matmul(
                        out=p2[:, :], lhsT=xot[:, ts(wi, D)],
                        rhs=iden_t[:, :], start=False, stop=True,
                    )
                    nc.scalar.activation(
                        out=osb[:, ts(wi, D)], in_=p2[:, :], func=relu,
                        scale=ab2_t[:, 0:1], bias=ab2_t[:, 1:2],
                    )

                # out-DMA rides the Activation HWDGE queue: it waits on
                # osb (end of this granule's compute), and on the Sync queue
                # it would head-block the next granule's input DMAs
                nc.scalar.dma_start(
                    out=out[:, g * WG * D:(g + 1) * WG * D], in_=osb[:, :]
                )

    nc.compile()
    return nc


def _get_nc(key):
    if key not in _NC_CACHE:
        _NC_CACHE[key] = _build()
    return _NC_CACHE[key]


# --------------------------------------------------------------- host driver
def _prepare(x, edge_index, edge_attr, eps, W1, b1, g1, bt1, rm1, rv1,
             W2, b2, g2, bt2, rm2, rv2):
    """Shard + reformat all inputs. Returns (in_maps, pos_of_node)."""
    src = np.asarray(edge_index[0], dtype=np.int64)
    dst = np.asarray(edge_index[1], dtype=np.int64)
    x = np.asarray(x, dtype=np.float32)
    ea_f = np.asarray(edge_attr, dtype=np.float32)

    pos_of, core_of = _plan_nodes(dst)
    x_bf = x.astype(BF16)

    # --- replicated constants
    epsf = float(np.asarray(eps))
    iotac = np.tile(np.arange(P, dtype=np.float32), (P, 1)).astype(BF16)
    ideps = ((1.0 + epsf) * np.eye(P)).astype(BF16)
    iden = np.eye(P, dtype=np.float32).astype(BF16)
    w1tm = np.ascontiguousarray(np.asarray(W1, np.float32).T).astype(BF16)
    w2tm = np.ascontiguousarray(np.asarray(W2, np.float32).T).astype(BF16)
    inv1 = 1.0 / np.sqrt(np.asarray(rv1, np.float32) + BN_EPS)
    a1 = np.asarray(g1, np.float32) * inv1
    beta1 = a1 * np.asarray(b1, np.float32) + np.asarray(bt1, np.float32) \
        - np.asarray(rm1, np.float32) * a1
    inv2 = 1.0 / np.sqrt(np.asarray(rv2, np.float32) + BN_EPS)
    a2 = np.asarray(g2, np.float32) * inv2
    beta2 = a2 * np.asarray(b2, np.float32) + np.asarray(bt2, np.float32) \
        - np.asarray(rm2, np.float32) * a2
    ab1 = np.ascontiguousarray(np.stack([a1, beta1], 1).astype(np.float32))
    ab2 = np.ascontiguousarray(np.stack([a2, beta2], 1).astype(np.float32))

    in_maps = []
    core_e = core_of[dst]
    for c in range(NCORES):
        em = core_e == c
        sc = src[em]
        posd = pos_of[dst[em]] - c * BPC
        wc = posd // P                      # window within core
        nrel = posd % P                     # node slot within window
        eac = ea_f[em]

        # edge -> slot: group by window (stable order)
        order_e = np.argsort(wc, kind="stable")
        ow = wc[order_e]
        counts = np.bincount(ow, minlength=NW)
        assert counts.max() <= WCAP
        starts = np.zeros(NW, np.int64)
        np.cumsum(counts[:-1], out=starts[1:])
        offs = np.arange(len(sc), dtype=np.int64) - starts[ow]
        k_o = offs // P
        q_o = offs % P
        col = ow * KB + k_o                 # global block col (0..NBC)

        so = sc[order_e]                    # src node per ordered edge
        # per-granule compact tables + chunk-local rows
        xgt_dev = np.zeros((NG * P, NELEM), dtype=np.float32)
        srcrow = np.zeros((NBC, P), np.int16)
        gstart = np.zeros(NG + 1, np.int64)
        wstarts = np.concatenate([starts, [len(so)]])
        for g in range(NG):
            gstart[g] = wstarts[g * WG]
        gstart[NG] = len(so)
        for g in range(NG):
            seg = slice(gstart[g], gstart[g + 1])
            U, inv = np.unique(so[seg], return_inverse=True)
            assert len(U) <= NELEM
            xgt_dev[g * P:(g + 1) * P, :len(U)] = \
                np.ascontiguousarray(x_bf[U].astype(np.float32).T)
            srcrow[col[seg], q_o[seg]] = inv.astype(np.int16)

        dstrel_c = np.full((NBC, P), -1.0, np.float32)
        dstrel_c[col, q_o] = nrel[order_e].astype(np.float32)
        ea_cd = np.zeros((NBC, P, D), dtype=BF16)
        ea_cd[col, q_o] = eac[order_e].astype(BF16)

        # idx per half-granule: flat i = block*128 + q ;
        # sbuf[p, s] = flat[s*16 + p%16], replicated over 8 groups of 16
        idx_cols = []
        for g in range(NG):
            for h in range(2):
                b0 = g * NBG + h * HB
                flat = srcrow[b0:b0 + HB].reshape(-1)        # [HB*128]
                idx_cols.append(flat.reshape(-1, 16).T)      # [16, HB*8]
        idx16 = np.concatenate(idx_cols, axis=1)             # [16, NIDX]
        idx_dev = np.ascontiguousarray(np.tile(idx16, (P // 16, 1)))

        dstrel_dev = np.ascontiguousarray(dstrel_c.T).astype(BF16)
        # feature-major: ea_dev[f, col*128 + q] = ea[edge at (col, q)][f]
        ea_dev = np.ascontiguousarray(
            ea_cd.transpose(2, 0, 1).reshape(P, NBC * P))

        # own-node rows (window-major, node-major partitions)
        xperm = np.zeros((NW, P, D), dtype=BF16)
        nodes_c = np.nonzero(core_of == c)[0]
        pc = pos_of[nodes_c] - c * BPC
        xperm[pc // P, pc % P] = x_bf[nodes_c]
        xo_dev = np.ascontiguousarray(
            xperm.transpose(1, 0, 2).reshape(P, NW * D))

        in_maps.append({
            "xgt": xgt_dev,
            "ea": ea_dev,
            "xo": xo_dev,
            "idx": idx_dev,
            "dstrel": dstrel_dev,
            "iotac": iotac,
            "ideps": ideps,
            "iden": iden,
            "w1t": w1tm,
            "w2t": w2tm,
            "ab1": ab1,
            "ab2": ab2,
        })
    return in_maps, pos_of


def kernel(**inputs) -> np.ndarray:
    global LAST_RESULTS
    x = np.asarray(inputs["x"], dtype=np.float32)
    assert x.shape == (N_NODES, D)

    in_maps, pos_of = _prepare(
        x, inputs["edge_index"], inputs["edge_attr_emb"], inputs["eps"],
        inputs["W1"], inputs["b1"], inputs["g1"], inputs["bt1"],
        inputs["rm1"], inputs["rv1"],
        inputs["W2"], inputs["b2"], inputs["g2"], inputs["bt2"],
        inputs["rm2"], inputs["rv2"],
    )
    nc = _get_nc("v3")
    res = run_bass_kernel_spmd(nc, in_maps, core_ids=list(range(NCORES)))
    LAST_RESULTS = res

    # out[c] is [P(feature), NW*P(node)]: col w*128+n -> padded node
    # row c*BPC + w*128 + n
    outp = np.stack([res.results[c]["out"] for c in range(NCORES)])
    out_rows = outp.astype(np.float32).reshape(NCORES, P, NW, P) \
        .transpose(0, 2, 3, 1).reshape(NPAD, D)
    return np.ascontiguousarray(out_rows[pos_of])


# revision 15
# speedup vs baseline: 1.1999x; 1.0025x over previous
"""Trainium2 Bass kernel for a GINE message-passing layer.

Reference computation (N=100000 nodes, E=600000 edges, D=128):
    msg  = relu(x[src] + edge_attr)            # [E, D]
    aggr = segment_sum(msg, dst, N)            # [N, D]
    z    = (1 + eps) * x + aggr
    h    = relu(bn1(z @ W1.T + b1)) @ W2.T + b2
    out  = relu(bn2(x + h))

Distribution strategy (8 NeuronCores, host-side shard/unshard):
  * Nodes are partitioned across the 8 cores (graph parallel) by a
    serpentine deal over in-degree-sorted nodes; within a core the same
    deal (plus a swap-repair pass) assigns nodes to 98 windows of 128 so
    every window receives at most 768 incoming edges (6 blocks of 128).
  * Edges are assigned to the core that owns their destination node, so
    the segment-sum is core-local.  The "halo" (src-node features) is
    materialized per (core, granule of 7 windows) as a compact
    feature-major bf16 table in HBM; each table is streamed sequentially
    into SBUF (SWDGE cast-DMA to f32) — no per-row DMA descriptors.
  * MLP weights / BN parameters are replicated (folded into per-feature
    affine scale+bias on the host; O(D) work).

Per-core device pipeline, bf16 activations (feature-major [feat, node]):
  1. per granule: stream the chunk table, then gather x[src] columns
     with GpSimd ap_gather (SBUF->SBUF, ~0.6ns/column — the SWDGE
     per-descriptor path costs ~8ns/row and was the old bottleneck),
  2. PE-transpose 3-block groups of the gathered feature-major columns
     into PSUM, VectorE adds the (edge-major) streamed edge_attr,
     ScalarE relu -> messages [edge, feat] in bf16,
  3. one-hot selection matrices S (VectorE iota-compare, bf16) turn the
     segment-sum into PE matmuls accumulated in PSUM:
         aggr[f, n] += sum_m msg[m, f] * S[m, n]
     plus an identity-matmul that adds (1+eps)*x (and transposes x to
     feature-major for free),
  4. MLP1 matmul + fused BN1+ReLU (ScalarE activation, per-partition
     affine), MLP2 matmul + identity-matmul residual + fused BN2+ReLU,
  5. output stays feature-major; the host transposes it back.
"""

import numpy as np
import ml_dtypes

import concourse.bass as bass
import concourse.bacc as bacc
import concourse.mybir as mybir
import concourse.tile as tile
from concourse.bass_utils import run_bass_kernel_spmd

# ---------------------------------------------------------------- constants
N_NODES = 100000
D = 128
P = 128                      # partitions
NCORES = 8
NW = 98                      # 128-node windows per core
BPC = NW * P                 # node slots per core (12544)
NPAD = NCORES * BPC          # padded node table rows (100352)
WG = 7                       # windows per granule (pipeline unit)
NG = NW // WG                # granules (14)
KB = 6                       # 128-edge blocks per window
WCAP = KB * P                # max in-edges per window (768)
NBG = WG * KB                # blocks per granule (42)
NBC = NW * KB                # blocks per core (588)
NELEM = NBG * P              # chunk-table rows per granule (5376)
HB = NBG // 2                # blocks per half-granule gather (21)
BN_EPS = 1e-5

BF16 = ml_dtypes.bfloat16

_NC_CACHE: dict = {}
LAST_RESULTS = None          # BassKernelResults of the most recent run


# ------------------------------------------------------------- host planning
def _serpentine(n, nbins):
    """Deal 0..n-1 into nbins bins, boustrophedon. Returns bin index."""
    g, o = np.divmod(np.arange(n), nbins)
    return np.where(g % 2 == 0, o, nbins - 1 - o)


def _plan_nodes(dst):
    """Serpentine deal of in-degree-sorted nodes to cores and windows,
    then swap-repair so every window has <= WCAP in-edges.
    Returns pos_of_node (global padded position)."""
    deg = np.bincount(dst, minlength=N_NODES)
    order = np.argsort(-deg, kind="stable")
    ranks = np.empty(N_NODES, np.int64)
    ranks[order] = np.arange(N_NODES)
    core_of = _serpentine(N_NODES, NCORES)[ranks]

    pos_of = np.empty(N_NODES, np.int64)
    for c in range(NCORES):
        nodes_c = order[core_of[order] == c]       # degree-desc within core
        nc_ = len(nodes_c)
        assert nc_ == N_NODES // NCORES and nc_ <= BPC
        w = _serpentine(nc_, NW)
        slot = np.arange(nc_) // NW        # swapped together with w below

        # swap-repair: windows must stay under WCAP in-edges
        dw = deg[nodes_c]
        cnt = np.bincount(w, weights=dw, minlength=NW).astype(np.int64)
        for _ in range(5000):
            hi = int(np.argmax(cnt))
            over = int(cnt[hi] - WCAP)
            if over <= 0:
                break
            done = False
            cand_hi = np.nonzero(w == hi)[0]
            dh = dw[cand_hi]
            for lo in np.argsort(cnt):
                lo = int(lo)
                slack = int(WCAP - cnt[lo])
                if slack <= 0 or lo == hi:
                    break
                t = min(over, slack)
                cand_lo = np.nonzero(w == lo)[0]
                dl = dw[cand_lo]
                dmat = dh[:, None] - dl[None, :]
                valid = (dmat >= 1) & (dmat <= slack)
                if valid.any():
                    score = np.where(valid, np.abs(dmat - t), 1 << 30)
                    ai, bi = np.unravel_index(np.argmin(score), score.shape)
                    a, b = cand_hi[ai], cand_lo[bi]
                    delta = int(dw[a] - dw[b])
                    w[a], w[b] = w[b], w[a]
                    slot[a], slot[b] = slot[b], slot[a]
                    cnt[hi] -= delta
                    cnt[lo] += delta
                    done = True
                    break
            if not done:
                raise RuntimeError("window repair failed")
        else:
            raise RuntimeError("window repair did not converge")
        assert cnt.max() <= WCAP
        pos_of[nodes_c] = c * BPC + w * P + slot
    return pos_of, core_of


# ------------------------------------------------------------- device build
def _build():
    """Build the per-core Bass program (SPMD: same program, per-core data)."""
    f32 = mybir.dt.float32
    bf16 = mybir.dt.bfloat16
    i16 = mybir.dt.int16
    NIDX = NG * (NBG * P // 16)          # idx columns (4704)

    nc = bacc.Bacc(None)
    xgt = nc.dram_tensor("xgt", [NG * P, NELEM], f32, kind="ExternalInput")
    ea = nc.dram_tensor("ea", [P, NBC * D], bf16, kind="ExternalInput")
    xo = nc.dram_tensor("xo", [P, NW * D], bf16, kind="ExternalInput")
    idx = nc.dram_tensor("idx", [P, NIDX], i16, kind="ExternalInput")
    dstrel = nc.dram_tensor("dstrel", [P, NBC], bf16, kind="ExternalInput")
    iotac = nc.dram_tensor("iotac", [P, P], bf16, kind="ExternalInput")
    ideps = nc.dram_tensor("ideps", [P, P], bf16, kind="ExternalInput")
    iden = nc.dram_tensor("iden", [P, P], bf16, kind="ExternalInput")
    w1t = nc.dram_tensor("w1t", [D, D], bf16, kind="ExternalInput")
    w2t = nc.dram_tensor("w2t", [D, D], bf16, kind="ExternalInput")
    ab1 = nc.dram_tensor("ab1", [D, 2], f32, kind="ExternalInput")
    ab2 = nc.dram_tensor("ab2", [D, 2], f32, kind="ExternalInput")
    out = nc.dram_tensor("out", [P, NW * D], bf16, kind="ExternalOutput")

    relu = mybir.ActivationFunctionType.Relu
    copyf = mybir.ActivationFunctionType.Copy
    iseq = mybir.AluOpType.is_equal
    addop = mybir.AluOpType.add
    ts = bass.ts
    CPG = NBG * P // 16                   # idx cols per granule (336)
    CPH = CPG // 2                        # idx cols per half-granule (168)
    NIH = HB * P                          # idxs per half-granule call (2688)

    with tile.TileContext(nc) as tc:
        with (
            tc.tile_pool(name="const", bufs=1) as cp,
            tc.tile_pool(name="xck", bufs=2) as xckp,
            tc.tile_pool(name="gx", bufs=2) as gxp,
            tc.tile_pool(name="msg", bufs=2) as msgp,
            tc.tile_pool(name="mfm", bufs=2) as mfmp,
            tc.tile_pool(name="ea", bufs=2) as eap,
            tc.tile_pool(name="sel", bufs=2) as sp,
            tc.tile_pool(name="xot", bufs=2) as xop,
            tc.tile_pool(name="z", bufs=2) as zp,
            tc.tile_pool(name="u", bufs=2) as up,
            tc.tile_pool(name="osb", bufs=2) as osp,
            tc.tile_pool(name="pt", bufs=2, space="PSUM") as ptp,
            tc.tile_pool(name="pz", bufs=1, space="PSUM") as pzp,
            tc.tile_pool(name="ph", bufs=1, space="PSUM") as php,
            tc.tile_pool(name="p2", bufs=1, space="PSUM") as p2p,
        ):
            # resident tensors
            idx_t = cp.tile([P, NIDX], i16)
            nc.sync.dma_start(out=idx_t[:, :], in_=idx[:, :])
            dst_t = cp.tile([P, NBC], bf16)
            nc.sync.dma_start(out=dst_t[:, :], in_=dstrel[:, :])
            iota_t = cp.tile([P, P], bf16)
            nc.sync.dma_start(out=iota_t[:, :], in_=iotac[:, :])
            ideps_t = cp.tile([P, P], bf16)
            nc.sync.dma_start(out=ideps_t[:, :], in_=ideps[:, :])
            iden_t = cp.tile([P, P], bf16)
            nc.sync.dma_start(out=iden_t[:, :], in_=iden[:, :])
            w1t_t = cp.tile([D, D], bf16)
            nc.sync.dma_start(out=w1t_t[:, :], in_=w1t[:, :])
            w2t_t = cp.tile([D, D], bf16)
            nc.sync.dma_start(out=w2t_t[:, :], in_=w2t[:, :])
            ab1_t = cp.tile([D, 2], f32)
            nc.sync.dma_start(out=ab1_t[:, :], in_=ab1[:, :])
            ab2_t = cp.tile([D, 2], f32)
            nc.sync.dma_start(out=ab2_t[:, :], in_=ab2[:, :])

            for g in range(NG):
                # ---- stream this granule's compact src table (f32)
                xck = xckp.tile([P, NELEM], f32)
                nc.sync.dma_start(
                    out=xck[:, :], in_=xgt[g * P:(g + 1) * P, :])
                # ---- edge attrs (edge-major) + own nodes
                ea_t = eap.tile([P, NBG * D], bf16)
                nc.sync.dma_start(
                    out=ea_t[:, :], in_=ea[:, g * NBG * D:(g + 1) * NBG * D])
                xot = xop.tile([P, WG * D], bf16)
                nc.sync.dma_start(
                    out=xot[:, :], in_=xo[:, g * WG * D:(g + 1) * WG * D])

                # ---- one-hot selection matrices for the whole granule
                sel = sp.tile([P, NBG * D], bf16)
                col0 = g * NBG
                in0 = (
                    dst_t[:, col0:col0 + NBG]
                    .rearrange("p (j o) -> p j o", o=1)
                    .to_broadcast([P, NBG, P])
                )
                in1 = (
                    iota_t[:, :]
                    .rearrange("p (o n) -> p o n", o=1)
                    .to_broadcast([P, NBG, P])
                )
                nc.vector.tensor_tensor(
                    out=sel[:, :].rearrange("p (j n) -> p j n", j=NBG),
                    in0=in0, in1=in1, op=iseq,
                )

                # ---- messages: gather columns (feature-major), +ea on
                # VectorE, PE-transpose 3-block groups, ScalarE relu does
                # the PSUM->SBUF move into edge-major msg
                msg = msgp.tile([P, NBG * D], bf16)
                mfms = []
                for h in range(2):
                    gx = gxp.tile([P, HB * D], f32)
                    nc.gpsimd.ap_gather(
                        gx[:, :].rearrange("p (n d) -> p n d", d=1),
                        xck[:, :].rearrange("p (n d) -> p n d", d=1),
                        idx_t[:, g * CPG + h * CPH:g * CPG + (h + 1) * CPH],
                        channels=P, num_elems=NELEM, d=1, num_idxs=NIH,
                    )
                    mfm = mfmp.tile([P, HB * D], bf16)
                    nc.vector.tensor_tensor(
                        out=mfm[:, :], in0=gx[:, :],
                        in1=ea_t[:, h * HB * D:(h + 1) * HB * D], op=addop,
                    )
                    mfms.append(mfm)
                for q in range(NBG // 6):
                    pt = ptp.tile([P, 6 * P], bf16, space="PSUM")
                    for i in range(6):
                        b = q * 6 + i
                        mf = mfms[b // HB]
                        nc.tensor.transpose(
                            pt[:, ts(i, P)],
                            mf[:, ts(b % HB, P)],
                            iden_t[:, :],
                        )
                    nc.scalar.activation(
                        out=msg[:, q * 6 * D:(q + 1) * 6 * D],
                        in_=pt[:, :], func=relu,
                    )

                osb = osp.tile([P, WG * D], bf16)
                # ---- aggregation, stage-major: all 7 windows into one
                # [P, 896] PSUM tile (no cross-engine ping-pong per window)
                pz = pzp.tile([P, WG * P], f32, space="PSUM")
                for wi in range(WG):
                    for k in range(KB):
                        b = wi * KB + k
                        nc.tensor.matmul(
                            out=pz[:, ts(wi, P)], lhsT=msg[:, ts(b, D)],
                            rhs=sel[:, ts(b, D)],
                            start=(k == 0), stop=False,
                        )
                    nc.tensor.matmul(
                        out=pz[:, ts(wi, P)], lhsT=xot[:, ts(wi, D)],
                        rhs=ideps_t[:, :], start=False, stop=True,
                    )
                z = zp.tile([P, WG * P], mybir.dt.bfloat16)
                nc.scalar.activation(out=z[:, :], in_=pz[:, :], func=copyf)

                # ---- MLP layer 1 + BN1 + relu (batched, N<=512)
                ph = php.tile([P, WG * P], f32, space="PSUM")
                nc.tensor.matmul(
                    out=ph[:, :512], lhsT=w1t_t[:, :], rhs=z[:, :512],
                    start=True, stop=True,
                )
                nc.tensor.matmul(
                    out=ph[:, 512:], lhsT=w1t_t[:, :], rhs=z[:, 512:],
                    start=True, stop=True,
                )
                u = up.tile([P, WG * P], mybir.dt.bfloat16)
                nc.scalar.activation(
                    out=u[:, :], in_=ph[:, :], func=relu,
                    scale=ab1_t[:, 0:1], bias=ab1_t[:, 1:2],
                )

                # ---- MLP layer 2 + residual + BN2 + relu
                p2 = p2p.tile([P, WG * P], f32, space="PSUM")
                nc.tensor.matmul(
                    out=p2[:, :512], lhsT=w2t_t[:, :], rhs=u[:, :512],
                    start=True, stop=False,
                )
                nc.tensor.matmul(
                    out=p2[:, 512:], lhsT=w2t_t[:, :], rhs=u[:, 512:],
                    start=True, stop=False,
                )
                for wi in range(WG):
                    nc.tensor.matmul(
                        out=p2[:, ts(wi, P)], lhsT=xot[:, ts(wi, D)],
                        rhs=iden_t[:, :], start=False, stop=True,
                    )
                nc.scalar.activation(
                    out=osb[:, :], in_=p2[:, :], func=relu,
                    scale=ab2_t[:, 0:1], bias=ab2_t[:, 1:2],
                )

                # out-DMA rides the Activation HWDGE queue: it waits on
                # osb (end of this granule's compute), and on the Sync queue
                # it would head-block the next granule's input DMAs
                nc.scalar.dma_start(
                    out=out[:, g * WG * D:(g + 1) * WG * D], in_=osb[:, :]
                )

    nc.compile()
    return nc


def _get_nc(key):
    if key not in _NC_CACHE:
        _NC_CACHE[key] = _build()
    return _NC_CACHE[key]


# --------------------------------------------------------------- host driver
def _prepare(x, edge_index, edge_attr, eps, W1, b1, g1, bt1, rm1, rv1,
             W2, b2, g2, bt2, rm2, rv2):
    """Shard + reformat all inputs. Returns (in_maps, pos_of_node)."""
    src = np.asarray(edge_index[0], dtype=np.int64)
    dst = np.asarray(edge_index[1], dtype=np.int64)
    x = np.asarray(x, dtype=np.float32)
    ea_f = np.asarray(edge_attr, dtype=np.float32)

    pos_of, core_of = _plan_nodes(dst)
    x_bf = x.astype(BF16)

    # --- replicated constants
    epsf = float(np.asarray(eps))
    iotac = np.tile(np.arange(P, dtype=np.float32), (P, 1)).astype(BF16)
    ideps = ((1.0 + epsf) * np.eye(P)).astype(BF16)
    iden = np.eye(P, dtype=np.float32).astype(BF16)
    w1tm = np.ascontiguousarray(np.asarray(W1, np.float32).T).astype(BF16)
    w2tm = np.ascontiguousarray(np.asarray(W2, np.float32).T).astype(BF16)
    inv1 = 1.0 / np.sqrt(np.asarray(rv1, np.float32) + BN_EPS)
    a1 = np.asarray(g1, np.float32) * inv1
    beta1 = a1 * np.asarray(b1, np.float32) + np.asarray(bt1, np.float32) \
        - np.asarray(rm1, np.float32) * a1
    inv2 = 1.0 / np.sqrt(np.asarray(rv2, np.float32) + BN_EPS)
    a2 = np.asarray(g2, np.float32) * inv2
    beta2 = a2 * np.asarray(b2, np.float32) + np.asarray(bt2, np.float32) \
        - np.asarray(rm2, np.float32) * a2
    ab1 = np.ascontiguousarray(np.stack([a1, beta1], 1).astype(np.float32))
    ab2 = np.ascontiguousarray(np.stack([a2, beta2], 1).astype(np.float32))

    in_maps = []
    core_e = core_of[dst]
    for c in range(NCORES):
        em = core_e == c
        sc = src[em]
        posd = pos_of[dst[em]] - c * BPC
        wc = posd // P                      # window within core
        nrel = posd % P                     # node slot within window
        eac = ea_f[em]

        # edge -> slot: group by window (stable order)
        order_e = np.argsort(wc, kind="stable")
        ow = wc[order_e]
        counts = np.bincount(ow, minlength=NW)
        assert counts.max() <= WCAP
        starts = np.zeros(NW, np.int64)
        np.cumsum(counts[:-1], out=starts[1:])
        offs = np.arange(len(sc), dtype=np.int64) - starts[ow]
        k_o = offs // P
        q_o = offs % P
        col = ow * KB + k_o                 # global block col (0..NBC)

        so = sc[order_e]                    # src node per ordered edge
        # per-granule compact tables + chunk-local rows
        xgt_dev = np.zeros((NG * P, NELEM), dtype=np.float32)
        srcrow = np.zeros((NBC, P), np.int16)
        gstart = np.zeros(NG + 1, np.int64)
        wstarts = np.concatenate([starts, [len(so)]])
        for g in range(NG):
            gstart[g] = wstarts[g * WG]
        gstart[NG] = len(so)
        for g in range(NG):
            seg = slice(gstart[g], gstart[g + 1])
            U, inv = np.unique(so[seg], return_inverse=True)
            assert len(U) <= NELEM
            xgt_dev[g * P:(g + 1) * P, :len(U)] = \
                np.ascontiguousarray(x_bf[U].astype(np.float32).T)
            srcrow[col[seg], q_o[seg]] = inv.astype(np.int16)

        dstrel_c = np.full((NBC, P), -1.0, np.float32)
        dstrel_c[col, q_o] = nrel[order_e].astype(np.float32)
        ea_cd = np.zeros((NBC, P, D), dtype=BF16)
        ea_cd[col, q_o] = eac[order_e].astype(BF16)

        # idx per half-granule: flat i = block*128 + q ;
        # sbuf[p, s] = flat[s*16 + p%16], replicated over 8 groups of 16
        idx_cols = []
        for g in range(NG):
            for h in range(2):
                b0 = g * NBG + h * HB
                flat = srcrow[b0:b0 + HB].reshape(-1)        # [HB*128]
                idx_cols.append(flat.reshape(-1, 16).T)      # [16, HB*8]
        idx16 = np.concatenate(idx_cols, axis=1)             # [16, NIDX]
        idx_dev = np.ascontiguousarray(np.tile(idx16, (P // 16, 1)))

        dstrel_dev = np.ascontiguousarray(dstrel_c.T).astype(BF16)
        # feature-major: ea_dev[f, col*128 + q] = ea[edge at (col, q)][f]
        ea_dev = np.ascontiguousarray(
            ea_cd.transpose(2, 0, 1).reshape(P, NBC * P))

        # own-node rows (window-major, node-major partitions)
        xperm = np.zeros((NW, P, D), dtype=BF16)
        nodes_c = np.nonzero(core_of == c)[0]
        pc = pos_of[nodes_c] - c * BPC
        xperm[pc // P, pc % P] = x_bf[nodes_c]
        xo_dev = np.ascontiguousarray(
            xperm.transpose(1, 0, 2).reshape(P, NW * D))

        in_maps.append({
            "xgt": xgt_dev,
            "ea": ea_dev,
            "xo": xo_dev,
            "idx": idx_dev,
            "dstrel": dstrel_dev,
            "iotac": iotac,
            "ideps": ideps,
            "iden": iden,
            "w1t": w1tm,
            "w2t": w2tm,
            "ab1": ab1,
            "ab2": ab2,
        })
    return in_maps, pos_of


def kernel(**inputs) -> np.ndarray:
    global LAST_RESULTS
    x = np.asarray(inputs["x"], dtype=np.float32)
    assert x.shape == (N_NODES, D)

    in_maps, pos_of = _prepare(
        x, inputs["edge_index"], inputs["edge_attr_emb"], inputs["eps"],
        inputs["W1"], inputs["b1"], inputs["g1"], inputs["bt1"],
        inputs["rm1"], inputs["rv1"],
        inputs["W2"], inputs["b2"], inputs["g2"], inputs["bt2"],
        inputs["rm2"], inputs["rv2"],
    )
    nc = _get_nc("v3")
    res = run_bass_kernel_spmd(nc, in_maps, core_ids=list(range(NCORES)))
    LAST_RESULTS = res

    # out[c] is [P(feature), NW*P(node)]: col w*128+n -> padded node
    # row c*BPC + w*128 + n
    outp = np.stack([res.results[c]["out"] for c in range(NCORES)])
    out_rows = outp.astype(np.float32).reshape(NCORES, P, NW, P) \
        .transpose(0, 2, 3, 1).reshape(NPAD, D)
    return np.ascontiguousarray(out_rows[pos_of])
